# revision 1
# baseline (speedup 1.0000x reference)
"""Trainium2 Bass kernel for CausalSelfAttention (GQA + QK-RMSNorm + RoPE + q_gain).

Sharding: 8-way tensor parallel over query heads. Core c owns q-heads
{2c, 2c+1} and recomputes kv-head c//2 (cheap). Each core produces a
partial output O_c @ Wo_c.T; the host sums the 8 partials.

Self-contained: hardcodes shapes from the problem spec
(x: [1, 4096, 2048], 16 heads / 4 kv heads, head_dim 128).
"""

import os
import numpy as np
from contextlib import ExitStack


def _setup_path():
    try:
        import concourse.bass  # noqa: F401
    except ImportError:
        import sys
        for p in ("/opt/trn_rl_repo", "/root/.axon_site/_ro/trn_rl_repo"):
            if os.path.isdir(p) and p not in sys.path:
                sys.path.insert(0, p)


_setup_path()

import concourse.bass as bass  # noqa: E402
import concourse.bacc as bacc  # noqa: E402
import concourse.mybir as mybir  # noqa: E402
import concourse.tile as tile  # noqa: E402
from concourse.bass_utils import run_bass_kernel_spmd  # noqa: E402

F32 = mybir.dt.float32
F32R = mybir.dt.float32r
BF = mybir.dt.bfloat16
import ml_dtypes
BF_NP = ml_dtypes.bfloat16
ACT = mybir.ActivationFunctionType

T = 4096
D = 2048
HD = 128
KD = D // 128           # 16 contraction tiles
NB = T // 512           # 8 sequence blocks of 512
EPS128 = 128.0 * 1.1920929e-07   # 128 * finfo(f32).eps
NEG = -1.0e30

# Module-level cache of the built program
_NC = None
LAST_RESULT = None


def _r(ap):
    return ap.bitcast(F32R)


def _rope_tables():
    ar = np.arange(0, HD, 2, dtype=np.float32) / np.float32(HD)
    inv = (np.float32(1.0) / (np.float32(10000.0) ** ar)).astype(np.float32)
    t = np.arange(T, dtype=np.float32)
    fr = np.outer(t, inv).astype(np.float32)          # [T, 64]
    cosT = np.ascontiguousarray(np.cos(fr).astype(np.float32).T)  # [64, T]
    sinT = np.ascontiguousarray(np.sin(fr).astype(np.float32).T)
    return cosT, sinT


def _bcast_ap(row_ap, parts=128):
    """Partition-broadcast read AP for a DRAM row of 512 elements."""
    return bass.AP(tensor=row_ap.tensor, offset=row_ap.offset,
                   ap=[[0, parts], [1, 512]])


def _scatter_ap(row_ap):
    """Read AP turning a DRAM row[512] into [128 part, 4]: out[p,j]=row[128j+p]."""
    return bass.AP(tensor=row_ap.tensor, offset=row_ap.offset,
                   ap=[[1, 128], [128, 4]])


def _build():
    nc = bacc.Bacc("TRN2")

    xT = nc.dram_tensor("xT", [D, T], BF, kind="ExternalInput")
    wq = nc.dram_tensor("wq", [128, KD, 256], BF, kind="ExternalInput")
    wk = nc.dram_tensor("wk", [128, KD, 128], BF, kind="ExternalInput")
    wv = nc.dram_tensor("wv", [128, KD, 128], BF, kind="ExternalInput")
    wo = nc.dram_tensor("wo", [128, 2, D], BF, kind="ExternalInput")
    g = nc.dram_tensor("g", [1, 2], F32, kind="ExternalInput")
    y = nc.dram_tensor("y", [T, D], F32, kind="ExternalOutput")

    cosT_np, sinT_np = _rope_tables()
    cos2_np = np.ascontiguousarray(np.concatenate([cosT_np, cosT_np], axis=0))
    sin2_np = np.ascontiguousarray(np.concatenate([sinT_np, sinT_np], axis=0))
    cos_d = nc.inline_tensor(cos2_np, "cosT")
    sin_d = nc.inline_tensor(sin2_np, "sinT")
    tri_np = np.where(np.arange(512)[None, :] >= np.arange(128)[:, None],
                      np.float32(0.0), np.float32(NEG)).astype(np.float32)
    tri_d = nc.inline_tensor(tri_np, "tri")          # [128, 512] mask for diag
    ones_d = nc.inline_tensor(np.ones((128, 1), BF_NP), "ones")
    id_d = nc.inline_tensor(np.eye(128).astype(BF_NP), "ident")

    with tile.TileContext(nc) as tc, ExitStack() as ctx:
        consts = ctx.enter_context(tc.tile_pool(name="consts", bufs=1))
        persist = ctx.enter_context(tc.tile_pool(name="persist", bufs=1))
        xt_pool = ctx.enter_context(tc.tile_pool(name="xt", bufs=18))
        qt_pool = ctx.enter_context(tc.tile_pool(name="qtp", bufs=3))
        sq_pool = ctx.enter_context(tc.tile_pool(name="sqp", bufs=3))
        stage_pool = ctx.enter_context(tc.tile_pool(name="stg", bufs=6))
        rt_pool = ctx.enter_context(tc.tile_pool(name="rtp", bufs=6))
        p_pool = ctx.enter_context(tc.tile_pool(name="ppool", bufs=6))
        o_pool = ctx.enter_context(tc.tile_pool(name="opool", bufs=3))
        y_pool = ctx.enter_context(tc.tile_pool(name="ypool", bufs=3))
        row_pool = ctx.enter_context(tc.tile_pool(name="rowp", bufs=12))
        bc_pool = ctx.enter_context(tc.tile_pool(name="bcp", bufs=6))
        vt_pool = ctx.enter_context(tc.tile_pool(name="vtp", bufs=2))
        cs_pool = ctx.enter_context(tc.tile_pool(name="csp", bufs=4))
        ps_proj = ctx.enter_context(tc.tile_pool(name="psproj", bufs=3, space="PSUM"))
        ps_st = ctx.enter_context(tc.tile_pool(name="psst", bufs=3, space="PSUM"))
        ps_pv = ctx.enter_context(tc.tile_pool(name="pspv", bufs=1, space="PSUM"))
        ps_sum = ctx.enter_context(tc.tile_pool(name="pssum", bufs=1, space="PSUM"))
        dram = ctx.enter_context(tc.tile_pool(name="dramp", bufs=1, space="DRAM"))

        # Resident constants
        wq_sb = consts.tile([128, KD, 256], BF, tag="wq")
        nc.sync.dma_start(out=wq_sb, in_=wq[:])
        wk_sb = consts.tile([128, KD, 128], BF, tag="wk")
        nc.sync.dma_start(out=wk_sb, in_=wk[:])
        wv_sb = consts.tile([128, KD, 128], BF, tag="wv")
        nc.sync.dma_start(out=wv_sb, in_=wv[:])
        wo_sb = consts.tile([128, 2, D], BF, tag="wo")
        nc.sync.dma_start(out=wo_sb, in_=wo[:])
        g_sb = consts.tile([1, 2], F32, tag="g")
        nc.sync.dma_start(out=g_sb, in_=g[:])
        tri_sb = consts.tile([128, 512], F32, tag="tri")
        nc.sync.dma_start(out=tri_sb, in_=tri_d[:])
        ones_sb = consts.tile([128, 1], BF, tag="ones")
        nc.sync.dma_start(out=ones_sb, in_=ones_d[:])
        id_sb = consts.tile([128, 128], BF, tag="id")
        nc.sync.dma_start(out=id_sb, in_=id_d[:])

        rows = dram.tile([NB, 5, 512], F32, tag="rows")

        kt_tiles = []
        v_tiles = []
        rk_tiles = []
        qt_tiles = [None] * NB
        o_tiles = [None] * NB
        for b in range(NB):
            kt_tiles.append(persist.tile([128, 512], BF, tag=f"kt{b}", name=f"kt{b}"))
            v_tiles.append(persist.tile([128, 4, 128], BF, tag=f"v{b}", name=f"v{b}"))
            rk_tiles.append(persist.tile([128, 4], F32, tag=f"rk{b}", name=f"rk{b}"))

        for b in range(NB):
            t0 = b * 512
            tsl = slice(t0, t0 + 512)

            # ---- Phase A: QKV projections (transposed orientation) ----
            xts = []
            for k in range(KD):
                xt = xt_pool.tile([128, 512], BF, tag="xt")
                nc.sync.dma_start(out=xt, in_=xT[k * 128:(k + 1) * 128, tsl])
                xts.append(xt)
            qa_ps = ps_proj.tile([128, 512], F32, tag="proj")
            qb_ps = ps_proj.tile([128, 512], F32, tag="proj")
            for k in range(KD):
                st_, sp_ = (k == 0), (k == KD - 1)
                nc.tensor.matmul(qa_ps, wq_sb[:, k, 0:128], xts[k],
                                 start=st_, stop=sp_)
                nc.tensor.matmul(qb_ps, wq_sb[:, k, 128:256], xts[k],
                                 start=st_, stop=sp_)
            kt_ps = ps_proj.tile([128, 512], F32, tag="proj")
            vt_ps = ps_proj.tile([128, 512], F32, tag="proj")
            for k in range(KD):
                st_, sp_ = (k == 0), (k == KD - 1)
                nc.tensor.matmul(kt_ps, wk_sb[:, k, :], xts[k],
                                 start=st_, stop=sp_)
                nc.tensor.matmul(vt_ps, wv_sb[:, k, :], xts[k],
                                 start=st_, stop=sp_)

            # Stage proj psums to SBUF so the psum slots free early and
            # block b+1's projections overlap block b's norm/rope chain.
            qa_st = stage_pool.tile([128, 512], F32, tag="stg")
            nc.scalar.copy(qa_st, qa_ps)
            qb_st = stage_pool.tile([128, 512], F32, tag="stg")
            nc.vector.tensor_copy(qb_st, qb_ps)
            kt_st = stage_pool.tile([128, 512], F32, tag="stg")
            nc.scalar.copy(kt_st, kt_ps)

            # ---- RMS-norm row factors ----
            # rq = gain / sqrt(ssq + 128*eps)   (scale 128^-0.5 cancels exactly)
            # rk = sqrt(128 / (ssq + 128*eps))
            rq_bc = [None, None]
            for ri, ps, kind, h in ((0, qa_st, 'q', 0), (1, qb_st, 'q', 1),
                                    (2, kt_st, 'k', None)):
                sq = sq_pool.tile([128, 512], BF, tag="sq")
                nc.scalar.activation(sq, ps, ACT.Square)
                ssq = ps_proj.tile([1, 512], F32, tag="proj")
                nc.tensor.matmul(ssq, ones_sb, sq)
                row = row_pool.tile([1, 512], F32, tag="row")
                nc.vector.tensor_scalar_add(row, ssq, EPS128)
                rec = row_pool.tile([1, 512], F32, tag="row")
                nc.vector.reciprocal(rec, row)
                fin = row_pool.tile([1, 512], F32, tag="row")
                if kind == 'q':
                    nc.scalar.activation(fin, rec, ACT.Sqrt)
                    fin2 = row_pool.tile([1, 512], F32, tag="row")
                    nc.vector.tensor_scalar_mul(fin2, fin, g_sb[:, h:h + 1])
                    nc.sync.dma_start(out=rows[b, ri], in_=fin2)
                    bc = bc_pool.tile([128, 512], F32, tag="bc")
                    nc.sync.dma_start(out=bc, in_=_bcast_ap(rows[b, ri]))
                    rq_bc[h] = bc
                else:
                    nc.scalar.activation(fin, rec, ACT.Sqrt, scale=128.0)
                    nc.sync.dma_start(out=rows[b, ri], in_=fin)
                    nc.sync.dma_start(out=rk_tiles[b], in_=_scatter_ap(rows[b, ri]))

            # ---- RoPE + rq application ----
            cos_b = cs_pool.tile([128, 512], F32, tag="cos")
            nc.sync.dma_start(out=cos_b, in_=cos_d[:, tsl])
            sin_b = cs_pool.tile([128, 512], F32, tag="sin")
            nc.sync.dma_start(out=sin_b, in_=sin_d[:, tsl])

            def rope(dst, src):
                # SB+SB operand pairs must share base partition; cos_b/sin_b
                # carry the table duplicated in both halves.
                t1 = rt_pool.tile([64, 512], F32, tag="rt")
                t2 = rt_pool.tile([64, 512], F32, tag="rt")
                nc.vector.tensor_mul(t1, src[0:64], cos_b[0:64])
                nc.vector.tensor_mul(t2, src[64:128], sin_b[64:128])
                nc.vector.tensor_add(dst[0:64], t1, t2)
                t3 = rt_pool.tile([64, 512], F32, tag="rt")
                t4 = rt_pool.tile([64, 512], F32, tag="rt")
                nc.vector.tensor_mul(t3, src[64:128], cos_b[64:128])
                nc.vector.tensor_mul(t4, src[0:64], sin_b[0:64])
                nc.vector.tensor_sub(dst[64:128], t3, t4)

            qt = qt_pool.tile([128, 2, 512], BF, tag="qt")
            qt_tiles[b] = qt
            for h in (0, 1):
                qtf = sq_pool.tile([128, 512], F32, tag="qtf")
                rope(qtf, qa_st if h == 0 else qb_st)
                nc.vector.tensor_mul(qt[:, h, :], qtf, rq_bc[h])
            rope(kt_tiles[b], kt_st)

            # ---- V transpose to [tk, hd] via PE ----
            vt_sb = vt_pool.tile([128, 512], BF, tag="vt")
            nc.vector.tensor_copy(vt_sb, vt_ps)
            for jj in range(4):
                tp = ps_st.tile([128, 128], BF, tag="st")
                nc.tensor.transpose(tp, vt_sb[:, jj * 128:(jj + 1) * 128], id_sb)
                nc.vector.tensor_copy(v_tiles[b][:, jj, :], tp)

            # ---- Phase B: attention for block b (both heads) ----
            o_sb = o_pool.tile([128, 2, 512], BF, tag="o")
            o_tiles[b] = o_sb
            nk = 4 * (b + 1)
            for h in (0, 1):
                pv_ps = ps_pv.tile([128, 512], F32, tag="pv")
                sum_ps = ps_sum.tile([1, 512], F32, tag="sums")
                qh = qt[:, h, :]
                for j in range(nk):
                    kb, ko = j // 4, (j % 4) * 128
                    st = ps_st.tile([128, 512], F32, tag="st")
                    nc.tensor.matmul(st, kt_tiles[kb][:, ko:ko + 128], qh)
                    p = p_pool.tile([128, 512], BF, tag="p")
                    scale_ap = rk_tiles[kb][:, (j % 4):(j % 4) + 1]
                    if j >= 4 * b:      # diagonal tile: causal mask
                        off = (j - 4 * b) * 128
                        nc.vector.tensor_add(st[:, off:off + 128],
                                             st[:, off:off + 128],
                                             tri_sb[:, 0:128])
                        if off:
                            nc.vector.memset(p[:, 0:off], 0.0)
                        nc.scalar.activation(p[:, off:512], st[:, off:512],
                                             ACT.Exp, scale=scale_ap)
                    else:
                        nc.scalar.activation(p, st, ACT.Exp, scale=scale_ap)
                    nc.tensor.matmul(pv_ps, v_tiles[kb][:, j % 4, :], p,
                                     start=(j == 0), stop=(j == nk - 1))
                    nc.tensor.matmul(sum_ps, ones_sb, p,
                                     start=(j == 0), stop=(j == nk - 1))
                rs = row_pool.tile([1, 512], F32, tag="row")
                nc.vector.reciprocal(rs, sum_ps)
                nc.sync.dma_start(out=rows[b, 3 + h], in_=rs)
                rs_bc = bc_pool.tile([128, 512], F32, tag="bc")
                nc.sync.dma_start(out=rs_bc, in_=_bcast_ap(rows[b, 3 + h]))
                nc.vector.tensor_mul(o_sb[:, h, :], pv_ps, rs_bc)

            # ---- Phase C: output projection (partial Y) ----
            for t4 in range(4):
                y_sb = y_pool.tile([128, D], F32, tag="y")
                for oc in range(4):
                    y_ps = ps_st.tile([128, 512], F32, tag="st")
                    for h in (0, 1):
                        nc.tensor.matmul(
                            y_ps,
                            o_sb[:, h, t4 * 128:(t4 + 1) * 128],
                            wo_sb[:, h, oc * 512:(oc + 1) * 512],
                            start=(h == 0), stop=(h == 1))
                    if oc % 2 == 0:
                        nc.vector.tensor_copy(y_sb[:, oc * 512:(oc + 1) * 512], y_ps)
                    else:
                        nc.scalar.copy(y_sb[:, oc * 512:(oc + 1) * 512], y_ps)
                nc.sync.dma_start(
                    out=y[t0 + t4 * 128: t0 + (t4 + 1) * 128, :], in_=y_sb)

    nc.finalize()
    return nc


def _get_nc():
    global _NC
    if _NC is None:
        _NC = _build()
    return _NC


def kernel(x, Wq, Wk, Wv, Wo, q_gain):
    global LAST_RESULT
    x = np.asarray(x, dtype=np.float32)
    Wq = np.asarray(Wq, dtype=np.float32)
    Wk = np.asarray(Wk, dtype=np.float32)
    Wv = np.asarray(Wv, dtype=np.float32)
    Wo = np.asarray(Wo, dtype=np.float32)
    q_gain = np.asarray(q_gain, dtype=np.float32)

    xT = np.ascontiguousarray(x[0].T).astype(BF_NP)         # [D, T]
    in_maps = []
    for c in range(8):
        h0, kv = 2 * c, c // 2
        wq_c = Wq[h0 * 128:(h0 + 2) * 128]                   # [256, D]
        wq_in = np.ascontiguousarray(
            wq_c.T.reshape(KD, 128, 256).transpose(1, 0, 2)).astype(BF_NP)
        wk_c = Wk[kv * 128:(kv + 1) * 128]                   # [128, D]
        wk_in = np.ascontiguousarray(
            wk_c.T.reshape(KD, 128, 128).transpose(1, 0, 2)).astype(BF_NP)
        wv_c = Wv[kv * 128:(kv + 1) * 128]
        wv_in = np.ascontiguousarray(
            wv_c.T.reshape(KD, 128, 128).transpose(1, 0, 2)).astype(BF_NP)
        wo_c = Wo[:, h0 * 128:(h0 + 2) * 128]                # [D, 256]
        wo_in = np.ascontiguousarray(
            wo_c.T.reshape(2, 128, D).transpose(1, 0, 2)).astype(BF_NP)
        g_in = np.ascontiguousarray(q_gain[h0:h0 + 2].reshape(1, 2))
        in_maps.append({"xT": xT, "wq": wq_in, "wk": wk_in, "wv": wv_in,
                        "wo": wo_in, "g": g_in})

    trace = bool(int(os.environ.get("KER_TRACE", "0")))
    res = run_bass_kernel_spmd(_get_nc(), in_maps, list(range(8)), trace=trace)
    LAST_RESULT = res
    acc = np.zeros((T, D), np.float64)
    for c in range(8):
        acc += res.results[c]["y"]
    return acc.astype(np.float32).reshape(1, T, D)



# revision 17
# speedup vs baseline: 1.2714x; 1.2714x over previous
"""Trainium2 Bass kernel for CausalSelfAttention (GQA + QK-RMSNorm + RoPE + q_gain).

Sharding: 8-way tensor parallel over query heads. Core c owns q-heads
{2c, 2c+1} and recomputes kv-head c//2 (cheap). Each core produces a
partial output O_c @ Wo_c.T in fp16; the host sums the 8 partials.

Self-contained: hardcodes shapes from the problem spec
(x: [1, 4096, 2048], 16 heads / 4 kv heads, head_dim 128).

Key design points (cost-model driven):
- All matmuls bf16 (1 cycle/row).  PE row count minimized:
  * softmax denominators via stationary-P matmuls with [128,1] output
    (cost ~1 row each instead of 512-row ones-matmuls)
  * V projected directly in [token, hd] orientation (x as stationary),
    killing the PE transposes
  * diagonal score/PV tiles truncated to the unmasked query range
- RMS norm factors via exp(-0.5*ln(ssq+eps)+bias) so only one activation
  table set (ln+exp) is ever loaded (sqrt lives in a different set and
  would force 2 table reloads per block).
- rk (key factors) consumed directly as per-partition exp scale; only the
  per-query factors (rq, 1/softmax-sum) take a DRAM broadcast roundtrip.
- Rope on DVE in bf16 (2x perf mode); causal mask as 0/1 multiply.
- One batched x-load and one batched fp16 y-store per 512-token block.
"""

import os
import numpy as np
from contextlib import ExitStack


def _setup_path():
    try:
        import concourse.bass  # noqa: F401
    except ImportError:
        import sys
        for p in ("/opt/trn_rl_repo", "/root/.axon_site/_ro/trn_rl_repo"):
            if os.path.isdir(p) and p not in sys.path:
                sys.path.insert(0, p)


_setup_path()

import concourse.bass as bass  # noqa: E402
import concourse.bacc as bacc  # noqa: E402
import concourse.mybir as mybir  # noqa: E402
import concourse.tile as tile  # noqa: E402
from concourse.bass_utils import run_bass_kernel_spmd  # noqa: E402

F32 = mybir.dt.float32
F16 = mybir.dt.float16
BF = mybir.dt.bfloat16
import ml_dtypes  # noqa: E402
BF_NP = ml_dtypes.bfloat16
ACT = mybir.ActivationFunctionType

T = 4096
D = 2048
HD = 128
KD = D // 128           # 16 contraction tiles
NB = T // 512           # 8 sequence blocks of 512
EPS128 = 128.0 * 1.1920929e-07   # 128 * finfo(f32).eps
HALF_LN128 = 0.5 * float(np.log(128.0))

# Module-level cache of the built program
_NC = None
LAST_RESULT = None


def _rope_tables():
    ar = np.arange(0, HD, 2, dtype=np.float32) / np.float32(HD)
    inv = (np.float32(1.0) / (np.float32(10000.0) ** ar)).astype(np.float32)
    t = np.arange(T, dtype=np.float32)
    fr = np.outer(t, inv).astype(np.float32)          # [T, 64]
    cosT = np.ascontiguousarray(np.cos(fr).astype(np.float32).T)  # [64, T]
    sinT = np.ascontiguousarray(np.sin(fr).astype(np.float32).T)
    return cosT, sinT


def _bcast_ap(row_ap, parts=128):
    """Partition-broadcast read AP for a DRAM row of 512 elements."""
    return bass.AP(tensor=row_ap.tensor, offset=row_ap.offset,
                   ap=[[0, parts], [1, 512]])


def _scatter_ap(row_ap):
    """DRAM-row AP [512] viewed as [128 part, 4]: row[128*s+p] <-> t[p, s]."""
    return bass.AP(tensor=row_ap.tensor, offset=row_ap.offset,
                   ap=[[1, 128], [128, 4]])


def _build():
    nc = bacc.Bacc("TRN2")

    xT = nc.dram_tensor("xT", [D, T], BF, kind="ExternalInput")
    wq = nc.dram_tensor("wq", [128, KD, 256], BF, kind="ExternalInput")
    wk = nc.dram_tensor("wk", [128, KD, 128], BF, kind="ExternalInput")
    wv = nc.dram_tensor("wv", [128, KD, 128], BF, kind="ExternalInput")
    wo = nc.dram_tensor("wo", [128, 2, D], BF, kind="ExternalInput")
    lng = nc.dram_tensor("lng", [1, 2], F32, kind="ExternalInput")  # ln(gain)
    y = nc.dram_tensor("y", [T, D], F16, kind="ExternalOutput")

    cosT_np, sinT_np = _rope_tables()
    cos2_np = np.concatenate([cosT_np, cosT_np], axis=0).astype(BF_NP)
    sin2_np = np.concatenate([sinT_np, sinT_np], axis=0).astype(BF_NP)
    cos_d = nc.inline_tensor(np.ascontiguousarray(cos2_np), "cosT")
    sin_d = nc.inline_tensor(np.ascontiguousarray(sin2_np), "sinT")
    # 0/1 causal mask for a 128x128 diagonal tile: key p visible to query q
    # (local coords) iff q >= p.
    tri_np = (np.arange(128)[None, :] >= np.arange(128)[:, None]).astype(BF_NP)
    tri_d = nc.inline_tensor(np.ascontiguousarray(tri_np), "tri01")
    ones_d = nc.inline_tensor(np.ones((128, 1), BF_NP), "ones")
    coefs_d = nc.inline_tensor(
        np.array([[EPS128, HALF_LN128]], dtype=np.float32), "coefs")

    with tile.TileContext(nc) as tc, ExitStack() as ctx:
        consts = ctx.enter_context(tc.tile_pool(name="consts", bufs=1))
        persist = ctx.enter_context(tc.tile_pool(name="persist", bufs=1))
        xt_pool = ctx.enter_context(tc.tile_pool(name="xt", bufs=2))
        stg_pool = ctx.enter_context(tc.tile_pool(name="stg", bufs=2))
        sq_pool = ctx.enter_context(tc.tile_pool(name="sqp", bufs=2))
        rt_pool = ctx.enter_context(tc.tile_pool(name="rtp", bufs=4))
        qt_pool = ctx.enter_context(tc.tile_pool(name="qtp", bufs=2))
        p_pool = ctx.enter_context(tc.tile_pool(name="ppool", bufs=8))
        o_pool = ctx.enter_context(tc.tile_pool(name="opool", bufs=2))
        y_pool = ctx.enter_context(tc.tile_pool(name="ypool", bufs=2))
        bc_pool = ctx.enter_context(tc.tile_pool(name="bcp", bufs=6))
        sm_pool = ctx.enter_context(tc.tile_pool(name="smp", bufs=4))
        ps_work = ctx.enter_context(tc.tile_pool(name="psw", bufs=1, space="PSUM"))
        ps_pv = ctx.enter_context(tc.tile_pool(name="pspv", bufs=1, space="PSUM"))
        dram = ctx.enter_context(tc.tile_pool(name="dramp", bufs=1, space="DRAM"))

        # First x block and Wq are loaded in interleaved k-chunks so the
        # first projection matmuls (which only need chunk 0 of each, via
        # subtile deps) start after ~2us instead of waiting for both full
        # loads on the exclusive DMA device.
        xt0 = xt_pool.tile([128, KD, 512], BF, tag="xt", name="xt0")
        wq_sb = consts.tile([128, KD, 256], BF, tag="wq")
        for k4 in range(0, KD, 4):
            nc.sync.dma_start(out=xt0[:, k4:k4 + 4, :], in_=bass.AP(
                tensor=xT, offset=k4 * 128 * T,
                ap=[[T, 128], [128 * T, 4], [1, 512]]))
            nc.sync.dma_start(out=wq_sb[:, k4:k4 + 4, :],
                              in_=wq[:, k4:k4 + 4, :])

        # Resident constants (ordered by when they are first needed)
        wk_sb = consts.tile([128, KD, 128], BF, tag="wk")
        nc.sync.dma_start(out=wk_sb, in_=wk[:])
        wv_sb = consts.tile([128, KD, 128], BF, tag="wv")
        nc.sync.dma_start(out=wv_sb, in_=wv[:])
        ones_sb = consts.tile([128, 1], BF, tag="ones")
        nc.sync.dma_start(out=ones_sb, in_=ones_d[:])
        coefs_bc = consts.tile([128, 2], F32, tag="coefs")
        nc.sync.dma_start(out=coefs_bc, in_=bass.AP(
            tensor=coefs_d, offset=0, ap=[[0, 128], [1, 2]]))
        lng_bc = consts.tile([128, 2], F32, tag="lng")
        nc.sync.dma_start(out=lng_bc, in_=bass.AP(
            tensor=lng, offset=0, ap=[[0, 128], [1, 2]]))
        tri_sb = consts.tile([128, 128], BF, tag="tri")
        nc.sync.dma_start(out=tri_sb, in_=tri_d[:])
        cos_sb = consts.tile([128, T], BF, tag="cos")
        nc.sync.dma_start(out=cos_sb, in_=cos_d[:])
        sin_sb = consts.tile([128, T], BF, tag="sin")
        nc.sync.dma_start(out=sin_sb, in_=sin_d[:])
        wo_sb = consts.tile([128, 2, D], BF, tag="wo")
        nc.sync.dma_start(out=wo_sb, in_=wo[:])

        rows = dram.tile([NB, 4, 512], F32, tag="rows")

        kt_tiles = []
        v_tiles = []
        rf_tiles = []
        for b in range(NB):
            kt_tiles.append(persist.tile([128, 512], BF, tag=f"kt{b}",
                                         name=f"kt{b}"))
            v_tiles.append(persist.tile([128, 4, 128], BF, tag=f"v{b}",
                                        name=f"v{b}"))
            # cols 0-3: rq head0, 4-7: rq head1, 8-11: rk
            rf_tiles.append(persist.tile([128, 12], F32, tag=f"rf{b}",
                                         name=f"rf{b}"))

        def rope(dst, src, tsl):
            # dst[0:64] = src[0:64]*cos + src[64:128]*sin
            # dst[64:128] = src[64:128]*cos - src[0:64]*sin
            # cos/sin tables duplicated in both partition halves so SB+SB
            # operand pairs share a base partition.  All bf16 (DVE 2x mode).
            cs0, cs1 = cos_sb[0:64, tsl], cos_sb[64:128, tsl]
            sn0, sn1 = sin_sb[0:64, tsl], sin_sb[64:128, tsl]
            t1 = rt_pool.tile([64, 512], BF, tag="rt", name="t1")
            t2 = rt_pool.tile([64, 512], BF, tag="rt", name="t2")
            nc.vector.tensor_mul(t1, src[0:64], cs0)
            nc.vector.tensor_mul(t2, src[64:128], sn1)
            nc.vector.tensor_add(dst[0:64], t1, t2)
            t3 = rt_pool.tile([64, 512], BF, tag="rt", name="t3")
            t4 = rt_pool.tile([64, 512], BF, tag="rt", name="t4")
            nc.vector.tensor_mul(t3, src[64:128], cs1)
            nc.vector.tensor_mul(t4, src[0:64], sn0)
            nc.vector.tensor_sub(dst[64:128], t3, t4)

        qt_tiles = [None] * NB
        o_tiles = [None] * NB
        qkst_tiles = [None] * NB
        rqbc_tiles = [None] * NB
        xts = {0: xt0}

        def emit_xt_prefetch(b):
            if b >= NB or b in xts:
                return
            xt = xt_pool.tile([128, KD, 512], BF, tag="xt", name="xt")
            nc.sync.dma_start(out=xt, in_=bass.AP(
                tensor=xT, offset=b * 512,
                ap=[[T, 128], [128 * T, KD], [1, 512]]))
            xts[b] = xt

        def proj_gen(b):
            """Emit projections + norm factors for block b as a generator.
            Each yield is ~one PE matmul so attention(b-1) can interleave
            this as PE filler while its Act exps are the bottleneck.
            Never uses more than 2 'w' psum slots at a time.  Act-engine ops
            (ln/rf) come last so they never block attention exps in the
            in-order Act stream."""
            xt = xts.pop(b)
            qk_st = stg_pool.tile([128, 3, 512], BF, tag="stg")
            qkst_tiles[b] = qk_st

            qa_ps = ps_work.tile([128, 512], F32, tag="proj", bufs=2, name="qa_ps")
            qb_ps = ps_work.tile([128, 512], F32, tag="proj", bufs=2, name="qb_ps")
            for k in range(KD):
                st_, sp_ = (k == 0), (k == KD - 1)
                nc.tensor.matmul(qa_ps, wq_sb[:, k, 0:128], xt[:, k, :],
                                 start=st_, stop=sp_)
                yield
                nc.tensor.matmul(qb_ps, wq_sb[:, k, 128:256], xt[:, k, :],
                                 start=st_, stop=sp_)
                yield
            nc.vector.tensor_copy(qk_st[:, 0, :], qa_ps)
            nc.vector.tensor_copy(qk_st[:, 1, :], qb_ps)

            kt_ps = ps_work.tile([128, 512], F32, tag="proj", bufs=2, name="kt_ps")
            v_ps = ps_work.tile([128, 4, 128], F32, tag="proj", bufs=2, name="v_ps")
            for k in range(KD):
                st_, sp_ = (k == 0), (k == KD - 1)
                nc.tensor.matmul(kt_ps, wk_sb[:, k, :], xt[:, k, :],
                                 start=st_, stop=sp_)
                yield
            # V in [token, hd] orientation: x as stationary, Wv as moving.
            for s in range(4):
                ss = slice(s * 128, (s + 1) * 128)
                for k in range(KD):
                    nc.tensor.matmul(v_ps[:, s, :], xt[:, k, ss],
                                     wv_sb[:, k, :],
                                     start=(k == 0), stop=(k == KD - 1))
                yield
            nc.vector.tensor_copy(qk_st[:, 2, :], kt_ps)
            nc.vector.tensor_copy(v_tiles[b][:], v_ps[:])

            # ssq[t] per target/subtile as [128,1] psum columns
            sq_st = sq_pool.tile([128, 3, 512], BF, tag="sq")
            nc.vector.tensor_mul(sq_st[:], qk_st[:], qk_st[:])
            ssq_ps = ps_work.tile([128, 512], F32, tag="proj", bufs=2, name="ssq_ps")
            for t in range(3):
                for s in range(4):
                    c = t * 4 + s
                    nc.tensor.matmul(ssq_ps[:, c:c + 1],
                                     sq_st[:, t, s * 128:(s + 1) * 128],
                                     ones_sb)
                yield
            # rq = g / sqrt(ssq+eps)  -> exp(-0.5*ln(ssq+eps) + ln g)
            # rk = sqrt(128/(ssq+eps)) -> exp(-0.5*ln(ssq+eps) + 0.5*ln 128)
            ln_t = sm_pool.tile([128, 12], F32, tag="sm", name="ln_t")
            nc.scalar.activation(ln_t, ssq_ps[:, 0:12], ACT.Ln,
                                 bias=coefs_bc[:, 0:1])
            rf = rf_tiles[b]
            nc.scalar.activation(rf[:, 0:4], ln_t[:, 0:4], ACT.Exp,
                                 scale=-0.5, bias=lng_bc[:, 0:1])
            nc.scalar.activation(rf[:, 4:8], ln_t[:, 4:8], ACT.Exp,
                                 scale=-0.5, bias=lng_bc[:, 1:2])
            nc.scalar.activation(rf[:, 8:12], ln_t[:, 8:12], ACT.Exp,
                                 scale=-0.5, bias=coefs_bc[:, 1:2])
            # rq broadcast roundtrip through DRAM
            rq_bc = [None, None]
            for h in (0, 1):
                nc.sync.dma_start(out=_scatter_ap(rows[b, h]),
                                  in_=rf[:, h * 4:(h + 1) * 4])
                bcb = bc_pool.tile([128, 512], F32, tag="bc", name="rqbc")
                nc.sync.dma_start(out=bcb, in_=_bcast_ap(rows[b, h]))
                rq_bc[h] = bcb
            rqbc_tiles[b] = rq_bc
            emit_xt_prefetch(b + 1)

        def emit_rope(b):
            # RoPE for block b (DVE only).  Emitted after y(b-1) so the rq
            # DMA roundtrip is covered by the y-projection matmuls.
            tsl = slice(b * 512, b * 512 + 512)
            qk_st = qkst_tiles[b]
            rq_bc = rqbc_tiles[b]
            qt = qt_pool.tile([128, 2, 512], BF, tag="qt")
            qt_tiles[b] = qt
            for h in (0, 1):
                rt = rt_pool.tile([128, 512], BF, tag="rtq", name="rt")
                rope(rt, qk_st[:, h, :], tsl)
                nc.vector.tensor_mul(qt[:, h, :], rt, rq_bc[h])
            rope(kt_tiles[b], qk_st[:, 2, :], tsl)

        def emit_attention(b, pump):
            # Software-pipelined over (j, h) steps: scores/exp run DEPTH
            # steps ahead of PV/sums, and `pump()` injects one next-block
            # projection matmul per step as PE filler (the attention inner
            # loop alone is Act-bound: exp 612ns vs PE ~440ns per step).
            qt = qt_tiles[b]
            nk = 4 * (b + 1)
            pv_ps = ps_pv.tile([128, 2, 512], F32, tag="pv")
            sums_ps = ps_work.tile([128, 512], F32, tag="sums", bufs=1, name="sums_ps")
            # The 8 per-(head,subtile) sum accumulators share one psum bank.
            # A matmul's start flag marks the WHOLE bank pending-zero, which
            # would clobber sibling columns' accumulation - so zero the bank
            # explicitly and use accumulate-only matmuls (start=False).
            nc.vector.memset(sums_ps[:, 0:8], 0.0)
            DEPTH = 3
            steps = [(j, h) for j in range(nk) for h in (0, 1)]
            stage = {}
            for step in range(len(steps) + DEPTH):
                if step < len(steps):
                    j, h = steps[step]
                    kb, jj = j // 4, j % 4
                    diag = (kb == b)
                    off = jj * 128 if diag else 0
                    sc = ps_work.tile([128, 512], F32, tag="sc", bufs=3, name="sc")
                    nc.tensor.matmul(sc[:, off:512],
                                     kt_tiles[kb][:, jj * 128:(jj + 1) * 128],
                                     qt[:, h, off:512])
                    ph = p_pool.tile([128, 512], BF, tag="p", name="ph")
                    nc.scalar.activation(ph[:, off:512], sc[:, off:512],
                                         ACT.Exp,
                                         scale=rf_tiles[kb][:, 8 + jj:9 + jj])
                    if diag:
                        nc.vector.tensor_mul(ph[:, off:off + 128],
                                             ph[:, off:off + 128], tri_sb)
                    stage[step] = (ph, kb, jj, diag, off, j, h)
                if step >= DEPTH:
                    ph, kb, jj, diag, off, j, h = stage.pop(step - DEPTH)
                    nc.tensor.matmul(pv_ps[:, h, off:512],
                                     v_tiles[kb][:, jj, :], ph[:, off:512],
                                     start=(j == 0), stop=(j == nk - 1),
                                     skip_group_check=True)
                    for s in range(jj if diag else 0, 4):
                        nc.tensor.matmul(
                            sums_ps[:, h * 4 + s:h * 4 + s + 1],
                            ph[:, s * 128:(s + 1) * 128], ones_sb,
                            start=False, stop=(diag and jj == s),
                            skip_group_check=True)
                pump()

            rs_sb = sm_pool.tile([128, 8], F32, tag="rs", name="rs_sb")
            nc.vector.reciprocal(rs_sb, sums_ps[:, 0:8])
            o_sb = o_pool.tile([128, 2, 512], BF, tag="o")
            o_tiles[b] = o_sb
            for h in (0, 1):
                nc.sync.dma_start(out=_scatter_ap(rows[b, 2 + h]),
                                  in_=rs_sb[:, h * 4:(h + 1) * 4])
                rsb = bc_pool.tile([128, 512], F32, tag="bc", name="rsbc")
                nc.sync.dma_start(out=rsb, in_=_bcast_ap(rows[b, 2 + h]))
                nc.vector.tensor_mul(o_sb[:, h, :], pv_ps[:, h, :], rsb)

        def y_gen(b):
            # ---- Phase C: output projection (partial Y, fp16) ----
            # Generator like proj_gen: usable as attention PE filler.  The
            # psum tiles share the fast-churn "sc" tag; copies are all DVE
            # (Act is the attention bottleneck); the store is split in two
            # halves so the copies of the second half overlap the first
            # half's DMA.
            t0 = b * 512
            o_sb = o_tiles[b]
            y_sb = y_pool.tile([128, 4, D], F16, tag="y")
            for t4 in range(4):
                for oc in range(4):
                    y_ps = ps_work.tile([128, 512], F32, tag="sc", bufs=3,
                                        name="y_ps")
                    nc.tensor.matmul(y_ps,
                                     o_sb[:, 0, t4 * 128:(t4 + 1) * 128],
                                     wo_sb[:, 0, oc * 512:(oc + 1) * 512],
                                     start=True, stop=False)
                    nc.tensor.matmul(y_ps,
                                     o_sb[:, 1, t4 * 128:(t4 + 1) * 128],
                                     wo_sb[:, 1, oc * 512:(oc + 1) * 512],
                                     start=False, stop=True)
                    nc.vector.tensor_copy(
                        y_sb[:, t4, oc * 512:(oc + 1) * 512], y_ps)
                    yield
                if t4 == 1:
                    nc.sync.dma_start(
                        out=bass.AP(tensor=y, offset=t0 * D,
                                    ap=[[D, 128], [128 * D, 2], [1, D]]),
                        in_=y_sb[:, 0:2, :])
            nc.sync.dma_start(
                out=bass.AP(tensor=y, offset=(t0 + 256) * D,
                            ap=[[D, 128], [128 * D, 2], [1, D]]),
                in_=y_sb[:, 2:4, :])

        def drain(gens):
            for g in gens:
                for _ in g:
                    pass

        def make_pump(gens):
            def pump():
                while gens:
                    try:
                        next(gens[0])
                        return
                    except StopIteration:
                        gens.pop(0)
            return pump

        # Per-iteration emission order:
        #   attention(b) [+ fillers: proj(b+1), then y(b-1)]
        #   -> remainder of fillers -> rope(b+1)
        drain([proj_gen(0)])
        emit_rope(0)
        ygen_prev = None
        for b in range(NB):
            fillers = []
            if b + 1 < NB:
                fillers.append(proj_gen(b + 1))
            if ygen_prev is not None:
                fillers.append(ygen_prev)
            emit_attention(b, make_pump(fillers))
            drain(fillers)
            if b + 1 < NB:
                emit_rope(b + 1)
            ygen_prev = y_gen(b)
        drain([ygen_prev])

    nc.finalize()
    return nc


def _get_nc():
    global _NC
    if _NC is None:
        _NC = _build()
    return _NC


def kernel(x, Wq, Wk, Wv, Wo, q_gain):
    global LAST_RESULT
    x = np.asarray(x, dtype=np.float32)
    Wq = np.asarray(Wq, dtype=np.float32)
    Wk = np.asarray(Wk, dtype=np.float32)
    Wv = np.asarray(Wv, dtype=np.float32)
    Wo = np.asarray(Wo, dtype=np.float32)
    q_gain = np.asarray(q_gain, dtype=np.float32)

    xT = np.ascontiguousarray(x[0].T).astype(BF_NP)         # [D, T]
    in_maps = []
    for c in range(8):
        h0, kv = 2 * c, c // 2
        wq_c = Wq[h0 * 128:(h0 + 2) * 128]                   # [256, D]
        wq_in = np.ascontiguousarray(
            wq_c.T.reshape(KD, 128, 256).transpose(1, 0, 2)).astype(BF_NP)
        wk_c = Wk[kv * 128:(kv + 1) * 128]                   # [128, D]
        wk_in = np.ascontiguousarray(
            wk_c.T.reshape(KD, 128, 128).transpose(1, 0, 2)).astype(BF_NP)
        wv_c = Wv[kv * 128:(kv + 1) * 128]
        wv_in = np.ascontiguousarray(
            wv_c.T.reshape(KD, 128, 128).transpose(1, 0, 2)).astype(BF_NP)
        wo_c = Wo[:, h0 * 128:(h0 + 2) * 128]                # [D, 256]
        wo_in = np.ascontiguousarray(
            wo_c.T.reshape(2, 128, D).transpose(1, 0, 2)).astype(BF_NP)
        lng_in = np.ascontiguousarray(
            np.log(q_gain[h0:h0 + 2]).reshape(1, 2).astype(np.float32))
        in_maps.append({"xT": xT, "wq": wq_in, "wk": wk_in, "wv": wv_in,
                        "wo": wo_in, "lng": lng_in})

    trace = bool(int(os.environ.get("KER_TRACE", "0")))
    res = run_bass_kernel_spmd(_get_nc(), in_maps, list(range(8)), trace=trace)
    LAST_RESULT = res
    acc = np.zeros((T, D), np.float64)
    for c in range(8):
        acc += res.results[c]["y"].astype(np.float64)
    return acc.astype(np.float32).reshape(1, T, D)
